# revision 12
# baseline (speedup 1.0000x reference)
"""Segmented softmax (segment_max/segment_sum normalize) on 8 Trainium2 cores.

Algorithm (per core, log-domain, no per-edge reciprocal):
  layout: partitions = 4 chunk-groups x 32 feature dims, free axis = edges.
  Host packs whole segments into bins of B edge-positions (one bin per
  (chunk, tile)), so no segment ever crosses a tile/chunk/core boundary and
  every tile is independent -> perfect SPMD, no cross-core reduction.

  per tile [128, B]:
    ex   = Exp(x)                                  (ACT)
    cs   = segmented-cumsum(ex)  via tensor_tensor_scan
           state = f*state + ex  (f=0 at segment starts)           (DVE)
    lcs  = Ln(cs)                                  (ACT)
    ltot = backward max-scan over reversed APs:
           state = (b + state) max lcs  (b=-BIG at segment ends)
           -> broadcasts ln(segment total) to every edge (cs is
           nondecreasing within a segment, so suffix-max works)    (DVE)
    y    = x - ltot                                (GPSIMD)
    out  = Exp(y)                                  (ACT)

  Flags f/b are host-computed per edge (bf16, tiny) and broadcast from
  4 chunk-rows to 128 partitions by a constant-matrix PE matmul into PSUM.

exp(x - ln(sum exp(x))) == exp(x - m)/sum(exp(x - m)) exactly in real
arithmetic; x ~ N(0,1) so exp(x) cannot overflow fp32.
"""

import sys

sys.path.insert(0, "/opt/trn_rl_repo")

from contextlib import ExitStack

import ml_dtypes
import numpy as np

import concourse.bacc as bacc
import concourse.bass as bass
import concourse.mybir as mybir
import concourse.tile as tile
from concourse.bass_utils import run_bass_kernel_spmd

N_CORES = 8
G = 4  # chunk groups per core (4 x 32 dims = 128 partitions)
D = 32
B = 1024  # edge positions per bin (= per chunk per tile)
NEG_BIG = -1e30
MM_N = 512  # max matmul free dim


def _pack_bins(idx: np.ndarray):
    """Greedily pack whole segments into bins of capacity B positions.

    Returns (starts, lens) per bin, padded so the bin count is a multiple
    of N_CORES * G.
    """
    E = idx.shape[0]
    change = np.flatnonzero(idx[1:] != idx[:-1]) + 1
    bnd = np.concatenate([[0], change, [E]]).astype(np.int64)
    seg_len = np.diff(bnd)
    assert seg_len.max() <= B, f"segment longer than bin: {seg_len.max()} > {B}"
    starts = []
    cur = 0
    while cur < E:
        j = np.searchsorted(bnd, cur + B, side="right") - 1
        end = int(bnd[j])
        assert end > cur
        starts.append(cur)
        cur = end
    starts.append(E)
    starts = np.asarray(starts, np.int64)
    lens = np.diff(starts)
    starts = starts[:-1]
    nb = len(starts)
    per = N_CORES * G
    T = -(-nb // per)  # tiles per chunk
    T += T % 2  # even tile count so tiles pair up
    pad = per * T - nb
    starts = np.concatenate([starts, np.full(pad, E, np.int64)])
    lens = np.concatenate([lens, np.zeros(pad, np.int64)])
    return starts, lens, T


def _build_nc(T: int, reps: int = 1) -> bass.Bass:
    L = T * B
    nc = bacc.Bacc("TRN2", target_bir_lowering=False, debug=False,
                   num_devices=N_CORES)
    xk = nc.dram_tensor("xk", [128, L], mybir.dt.float32,
                        kind="ExternalInput").ap()
    fb = nc.dram_tensor("fb", [2 * G, L], mybir.dt.bfloat16,
                        kind="ExternalInput").ap()
    sel = nc.dram_tensor("sel", [2 * G, 256], mybir.dt.bfloat16,
                         kind="ExternalInput").ap()
    yk = nc.dram_tensor("yk", [128, L], mybir.dt.float32,
                        kind="ExternalOutput").ap()

    AF = mybir.ActivationFunctionType
    OP = mybir.AluOpType

    with tile.TileContext(nc) as tc, ExitStack() as ctx:
        selp = ctx.enter_context(tc.tile_pool(name="selp", bufs=1))
        xp = ctx.enter_context(tc.tile_pool(name="xp", bufs=4))
        fbp = ctx.enter_context(tc.tile_pool(name="fbp", bufs=4))
        exp_p = ctx.enter_context(tc.tile_pool(name="exp", bufs=2))
        csp = ctx.enter_context(tc.tile_pool(name="csp", bufs=2))
        lcsp = ctx.enter_context(tc.tile_pool(name="lcsp", bufs=2))
        ltp = ctx.enter_context(tc.tile_pool(name="ltp", bufs=2))
        yp = ctx.enter_context(tc.tile_pool(name="yp", bufs=2))
        outp = ctx.enter_context(tc.tile_pool(name="outp", bufs=3))
        psf = ctx.enter_context(tc.tile_pool(name="psf", bufs=2, space="PSUM"))
        psb = ctx.enter_context(tc.tile_pool(name="psb", bufs=2, space="PSUM"))

        selt = selp.tile([2 * G, 256], mybir.dt.bfloat16)
        nc.sync.dma_start(selt[:], sel[:])

        P2 = 2 * B  # pair width: DMA/ACT/Pool ops run on tile pairs

        def body(p):
            sl = bass.ts(p, P2)
            xt = xp.tile([128, P2], mybir.dt.float32)
            nc.sync.dma_start(xt[:], xk[:, sl])
            fbt = fbp.tile([2 * G, P2], mybir.dt.bfloat16)
            nc.sync.dma_start(fbt[:], fb[:, sl])

            ext = exp_p.tile([128, P2], mybir.dt.float32)
            nc.scalar.activation(ext[:], xt[:], AF.Exp)

            # per 1024-subtile: flag broadcast (PE) + forward scan (DVE)
            cst = csp.tile([128, P2], mybir.dt.float32)
            pfs, pbs = [], []
            for k in range(2):
                pf = psf.tile([128, B], mybir.dt.float32)
                pb = psb.tile([128, B], mybir.dt.float32)
                off = k * B
                step = min(B, MM_N)
                for h in range(B // step):
                    hs = slice(off + h * step, off + (h + 1) * step)
                    nc.tensor.matmul(pf[:, bass.ts(h, step)],
                                     selt[:, 0:128], fbt[:, hs],
                                     start=True, stop=True)
                for h in range(B // step):
                    hs = slice(off + h * step, off + (h + 1) * step)
                    nc.tensor.matmul(pb[:, bass.ts(h, step)],
                                     selt[:, 128:256], fbt[:, hs],
                                     start=True, stop=True)
                nc.vector.tensor_tensor_scan(
                    cst[:, off:off + B], pf[:], ext[:, off:off + B], 0.0,
                    op0=OP.mult, op1=OP.add)
                pfs.append(pf)
                pbs.append(pb)

            lcst = lcsp.tile([128, P2], mybir.dt.float32)
            nc.scalar.activation(lcst[:], cst[:], AF.Ln)

            ltt = ltp.tile([128, P2], mybir.dt.float32)
            for k in range(2):
                off = k * B
                nc.vector.tensor_tensor_scan(
                    ltt[:, off:off + B][:, ::-1], pbs[k][:, ::-1],
                    lcst[:, off:off + B][:, ::-1], 0.0,
                    op0=OP.add, op1=OP.max)

            yt = yp.tile([128, P2], mybir.dt.float32)
            nc.gpsimd.tensor_sub(yt[:], xt[:], ltt[:])

            ot = outp.tile([128, P2], mybir.dt.float32)
            nc.scalar.activation(ot[:], yt[:], AF.Exp)
            nc.sync.dma_start(yk[:, sl], ot[:])

        assert T % 2 == 0
        if reps == 1:
            for p in range(T // 2):
                body(p)
        else:
            with tc.For_i(0, reps, 1):
                for p in range(T // 2):
                    body(p)
    nc.compile()
    return nc


def _make_sel() -> np.ndarray:
    sel = np.zeros((2 * G, 256), np.float32)
    for g in range(G):
        sel[g, g * 32:(g + 1) * 32] = 1.0
        sel[G + g, 128 + g * 32:128 + (g + 1) * 32] = 1.0
    return sel.astype(ml_dtypes.bfloat16)


def _prepare(edge_vec: np.ndarray, selected_edges: np.ndarray):
    """Host-side shard/pack. Returns (in_maps, src, mask, T)."""
    E = edge_vec.shape[0]
    idx = selected_edges[:, -2].astype(np.int64)
    starts, lens, T = _pack_bins(idx)
    NB = N_CORES * G * T

    ar = np.arange(B, dtype=np.int64)
    src = starts[:, None] + ar[None, :]  # [NB, B]
    mask = ar[None, :] < lens[:, None]
    np.copyto(src, -1, where=~mask)

    srcc = np.clip(src, 0, E - 1)
    idxs = np.where(mask, idx[srcc], -1)  # [NB, B]

    same_prev = np.zeros_like(mask)
    same_prev[:, 1:] = (idxs[:, 1:] == idxs[:, :-1]) & (idxs[:, 1:] >= 0)
    f_flag = same_prev.astype(np.float32)
    same_next = np.zeros_like(mask)
    same_next[:, :-1] = (idxs[:, :-1] == idxs[:, 1:]) & (idxs[:, :-1] >= 0)
    b_flag = np.where(same_next, np.float32(0.0), np.float32(NEG_BIG))

    xg = edge_vec[srcc]  # [NB, B, D]
    xg[~mask] = 0.0
    # [cores, G, T, B, D] -> [cores, G, D, T, B] -> [cores, 128, T*B]
    xk = np.ascontiguousarray(
        xg.reshape(N_CORES, G, T, B, D).transpose(0, 1, 4, 2, 3)
    ).reshape(N_CORES, 128, T * B)

    ff = f_flag.reshape(N_CORES, G, T * B)
    bf = b_flag.reshape(N_CORES, G, T * B)
    fbh = np.concatenate([ff, bf], axis=1).astype(ml_dtypes.bfloat16)

    sel = _make_sel()
    in_maps = [
        {"xk": xk[c], "fb": fbh[c], "sel": sel} for c in range(N_CORES)
    ]
    return in_maps, src, mask, T


def _unshard(results, src, mask, E, T):
    y_all = np.stack([results[c]["yk"] for c in range(N_CORES)])
    yg = (
        y_all.reshape(N_CORES, G, D, T, B)
        .transpose(0, 1, 3, 4, 2)
        .reshape(-1, B, D)
    )
    out = np.empty((E, D), np.float32)
    out[src[mask]] = yg[mask]
    return out


def kernel(edge_vec: np.ndarray, selected_edges: np.ndarray) -> np.ndarray:
    edge_vec = np.ascontiguousarray(np.asarray(edge_vec, np.float32))
    selected_edges = np.asarray(selected_edges)
    E = edge_vec.shape[0]

    in_maps, src, mask, T = _prepare(edge_vec, selected_edges)
    nc = _build_nc(T)
    res = run_bass_kernel_spmd(nc, in_maps, list(range(N_CORES)))
    global LAST_RESULTS
    LAST_RESULTS = res
    return _unshard(res.results, src, mask, E, T)


LAST_RESULTS = None
